# revision 2
# baseline (speedup 1.0000x reference)
"""Trainium2 Bass kernel for batched linear attention (no softmax).

Reference computation (per batch b):
    q = x Wq^T + bq ; k = x Wk^T + bk ; v = x Wv^T + bv
    out = (q k^T / sqrt(D)) v

Since there is no softmax, matmul associativity gives
    out = q (k^T v) / sqrt(D)
and with the augmented matrices x' = [x | 1], Aw = [W^T ; b] (so w = x' Aw):
    k^T v = Ak^T (x'^T x') Av = Ak^T G' Av
    out   = x' (Aq* (Ak^T G' Av))          with Aq* = Aq / sqrt(D)

which replaces the two S x S matmuls (dominant cost) with [D+1]-sized ones.
G' is symmetric, so only the upper trapezoid is computed on the PE and the
lower blocks are filled by PE transposes.

Sharding: 8 cores = 4 batches x 2 S-halves. Each core computes G' for its
full batch (pair-redundant) and the output projection only for its S-half.
All weight-layout prep (transposes, augmentation, scale folding) happens on
the host; the device does all O(S) FLOPs.
"""

import math

import numpy as np

B, S, D = 4, 4096, 768
DA = D + 1          # augmented dim (ones column / bias row)
P = 128
SH = S // 2         # per-core sequence half
N_CORES = 8
NT_S = S // P       # 32 sequence tiles for G'
ND = D // P         # 6 blocks of 128 over D
NT_SH = SH // P     # 16 output row blocks
CH_D = [(0, 512), (512, 256)]    # free-dim chunks covering 768
CH_DA = [(0, 512), (512, 257)]   # free-dim chunks covering 769

# Build-time configuration. "mm_dt": "f32" (safe) or "f32r" (4x faster PE,
# reduced-precision fp32 mode). "sym_g": exploit G' symmetry.
CONFIG = {"mm_dt": "f32", "sym_g": True}

_CACHE = {}


def _chunks(c0, c1, step=512):
    out = []
    while c0 < c1:
        w = min(step, c1 - c0)
        out.append((c0, w))
        c0 += w
    return out


def _build_nc(mm_dt="f32", sym_g=True):
    import concourse.bacc as bacc
    import concourse.mybir as mybir
    import concourse.tile as tile
    from concourse.masks import make_identity

    f32 = mybir.dt.float32
    if mm_dt == "f32":
        cast = lambda ap: ap
    elif mm_dt == "f32r":
        cast = lambda ap: ap.bitcast(mybir.dt.float32r)
    else:
        raise ValueError(mm_dt)

    nc = bacc.Bacc("TRN2", target_bir_lowering=False, debug=False,
                   num_devices=N_CORES)

    xa_t = nc.dram_tensor("xa", [S, DA], f32, kind="ExternalInput")
    xt_t = nc.dram_tensor("xt", [D, SH], f32, kind="ExternalInput")
    ak_t = nc.dram_tensor("ak", [DA, D], f32, kind="ExternalInput")
    av_t = nc.dram_tensor("av", [DA, D], f32, kind="ExternalInput")
    aqt_t = nc.dram_tensor("aqt", [D, DA], f32, kind="ExternalInput")
    out_t = nc.dram_tensor("out", [SH, D], f32, kind="ExternalOutput")
    xa, xt, ak, av, aqt, outd = (t.ap() for t in
                                 (xa_t, xt_t, ak_t, av_t, aqt_t, out_t))

    def mm(ps, lh, rh, start, stop):
        nc.tensor.matmul(ps, lhsT=cast(lh), rhs=cast(rh), start=start,
                         stop=stop)

    with tile.TileContext(nc) as tc:
        with tc.tile_pool(name="persist", bufs=1) as pp:
            # G' rows d'<768 stored as 6 partition-tiles side by side:
            # g_sb[p, t*DA + j] = G'[t*128 + p, j]
            g_sb = pp.tile([P, ND * DA], f32, name="g_sb", tag="g_sb")
            g_row = pp.tile([1, DA], f32, name="g_row", tag="g_row")
            at_sb = pp.tile([P, ND * D], f32, name="at_sb", tag="at_sb")
            at_row = pp.tile([1, D], f32, name="at_row", tag="at_row")
            kv_sb = pp.tile([P, ND * D], f32, name="kv_sb", tag="kv_sb")
            p2_sb = pp.tile([P, ND * D], f32, name="p2_sb", tag="p2_sb")
            p2_row = pp.tile([1, D], f32, name="p2_row", tag="p2_row")
            ident = pp.tile([P, P], f32, name="ident", tag="ident")
            ones_row = pp.tile([1, P], f32, name="ones_row", tag="ones_row")
            make_identity(nc, ident)
            nc.any.memset(ones_row[0:1, :], 1.0)

            # ---- Stage 1: G' = x'^T x' ----
            with tc.tile_pool(name="xp", bufs=1) as xp, \
                 tc.tile_pool(name="gps", bufs=6, space="PSUM") as gpsp:
                x_tiles = []
                for i in range(NT_S):
                    t = xp.tile([P, DA], f32, name=f"x{i}", tag=f"x{i}")
                    nc.sync.dma_start(out=t[:, :], in_=xa[i * P:(i + 1) * P, :])
                    x_tiles.append(t)

                if sym_g:
                    # upper trapezoid only: rows md*128..+128, cols md*128..769
                    jobs = []
                    for md in range(ND):
                        jobs += [(md, c0, cw) for (c0, cw)
                                 in _chunks(md * P, DA)]
                    passes = [[j for j in jobs if j[0] < 3],
                              [j for j in jobs if j[0] >= 3]]
                else:
                    passes = [[(md, c0, cw) for md in range(ND)
                               for (c0, cw) in [ch]] for ch in CH_DA]

                for pass_jobs in passes:
                    pss = {}
                    for (md, c0, cw) in pass_jobs:
                        pss[(md, c0)] = gpsp.tile(
                            [P, 512], f32, name=f"gps_{md}_{c0}", tag="gps")
                    for st in range(NT_S):
                        for (md, c0, cw) in pass_jobs:
                            mm(pss[(md, c0)][:, :cw],
                               x_tiles[st][:, md * P:(md + 1) * P],
                               x_tiles[st][:, c0:c0 + cw],
                               start=(st == 0), stop=(st == NT_S - 1))
                    for (md, c0, cw) in pass_jobs:
                        nc.vector.tensor_copy(
                            g_sb[:, md * DA + c0: md * DA + c0 + cw],
                            pss[(md, c0)][:, :cw])

            # mirror lower-triangle blocks + m-row
            with tc.tile_pool(name="tps", bufs=4, space="PSUM") as tpsp:
                if sym_g:
                    for md in range(1, ND):
                        for nb in range(md):
                            pt = tpsp.tile([P, 512], f32,
                                           name=f"tm{md}_{nb}", tag="tps")
                            nc.tensor.matmul(
                                pt[:, 0:P],
                                lhsT=g_sb[:, nb * DA + md * P:
                                          nb * DA + (md + 1) * P],
                                rhs=ident[:, :], is_transpose=True,
                                start=True, stop=True)
                            nc.vector.tensor_copy(
                                g_sb[:, md * DA + nb * P:
                                     md * DA + (nb + 1) * P],
                                pt[:, 0:P])
                # m-row: G'[768, :768] = transpose of col G'[:768, 768]
                for t in range(ND):
                    pr = tpsp.tile([P, 512], f32, name=f"tp{t}", tag="tps")
                    nc.tensor.matmul(
                        pr[0:1, 0:P],
                        lhsT=g_sb[:, t * DA + 768: t * DA + 769],
                        rhs=ident[:, :], is_transpose=True,
                        start=True, stop=True)
                    nc.vector.tensor_copy(g_row[0:1, t * P:(t + 1) * P],
                                          pr[0:1, 0:P])
                nc.any.memset(g_row[0:1, 768:769], float(S))

            # ---- Stage 2: AT' = G' Ak  ([769, 768], f' on partitions) ----
            with tc.tile_pool(name="akp", bufs=1) as akp, \
                 tc.tile_pool(name="ps2", bufs=4, space="PSUM") as ps2:
                ak_sb = akp.tile([P, ND * D], f32, name="ak_sb", tag="ak_sb")
                ak_row = akp.tile([1, D], f32, name="ak_row", tag="ak_row")
                for kt in range(ND):
                    nc.sync.dma_start(out=ak_sb[:, kt * D:(kt + 1) * D],
                                      in_=ak[kt * P:(kt + 1) * P, :])
                nc.sync.dma_start(out=ak_row[0:1, :], in_=ak[768:769, :])
                for mb in range(ND):
                    for (c0, cw) in CH_D:
                        ps = ps2.tile([P, 512], f32, name=f"atps{mb}_{c0}",
                                      tag="ps2")
                        for kt in range(ND + 1):
                            if kt < ND:
                                lh = g_sb[:, kt * DA + mb * P:
                                          kt * DA + (mb + 1) * P]
                                rh = ak_sb[:, kt * D + c0: kt * D + c0 + cw]
                            else:
                                lh = g_row[0:1, mb * P:(mb + 1) * P]
                                rh = ak_row[0:1, c0:c0 + cw]
                            mm(ps[:, :cw], lh, rh,
                               start=(kt == 0), stop=(kt == ND))
                        nc.vector.tensor_copy(
                            at_sb[:, mb * D + c0: mb * D + c0 + cw],
                            ps[:, :cw])
                for (c0, cw) in CH_D:   # AT' row f' = 768
                    ps = ps2.tile([P, 512], f32, name=f"atr{c0}", tag="ps2")
                    for kt in range(ND + 1):
                        if kt < ND:
                            lh = g_sb[:, kt * DA + 768: kt * DA + 769]
                            rh = ak_sb[:, kt * D + c0: kt * D + c0 + cw]
                        else:
                            lh = g_row[0:1, 768:769]
                            rh = ak_row[0:1, c0:c0 + cw]
                        mm(ps[0:1, :cw], lh, rh,
                           start=(kt == 0), stop=(kt == ND))
                    nc.vector.tensor_copy(at_row[0:1, c0:c0 + cw],
                                          ps[0:1, :cw])

            # ---- Stage 3: kv = AT'^T Av  ([768, 768], e on partitions) ----
            with tc.tile_pool(name="avp", bufs=1) as avp, \
                 tc.tile_pool(name="ps3", bufs=4, space="PSUM") as ps3:
                av_sb = avp.tile([P, ND * D], f32, name="av_sb", tag="av_sb")
                av_row = avp.tile([1, D], f32, name="av_row", tag="av_row")
                for kt in range(ND):
                    nc.sync.dma_start(out=av_sb[:, kt * D:(kt + 1) * D],
                                      in_=av[kt * P:(kt + 1) * P, :])
                nc.sync.dma_start(out=av_row[0:1, :], in_=av[768:769, :])
                for mb in range(ND):
                    for (c0, cw) in CH_D:
                        ps = ps3.tile([P, 512], f32, name=f"kvps{mb}_{c0}",
                                      tag="ps3")
                        for kt in range(ND + 1):
                            if kt < ND:
                                lh = at_sb[:, kt * D + mb * P:
                                           kt * D + (mb + 1) * P]
                                rh = av_sb[:, kt * D + c0: kt * D + c0 + cw]
                            else:
                                lh = at_row[0:1, mb * P:(mb + 1) * P]
                                rh = av_row[0:1, c0:c0 + cw]
                            mm(ps[:, :cw], lh, rh,
                               start=(kt == 0), stop=(kt == ND))
                        nc.vector.tensor_copy(
                            kv_sb[:, mb * D + c0: mb * D + c0 + cw],
                            ps[:, :cw])

            # ---- Stage 4: P2 = Aq* kv  ([769, 768], d' on partitions) ----
            with tc.tile_pool(name="aqp", bufs=1) as aqp, \
                 tc.tile_pool(name="ps4", bufs=4, space="PSUM") as ps4:
                aqt_sb = aqp.tile([P, ND * DA], f32, name="aqt_sb",
                                  tag="aqt_sb")
                for kt in range(ND):
                    nc.sync.dma_start(out=aqt_sb[:, kt * DA:(kt + 1) * DA],
                                      in_=aqt[kt * P:(kt + 1) * P, :])
                for mb in range(ND):
                    for (c0, cw) in CH_D:
                        ps = ps4.tile([P, 512], f32, name=f"p2ps{mb}_{c0}",
                                      tag="ps4")
                        for kt in range(ND):
                            mm(ps[:, :cw],
                               aqt_sb[:, kt * DA + mb * P:
                                      kt * DA + (mb + 1) * P],
                               kv_sb[:, kt * D + c0: kt * D + c0 + cw],
                               start=(kt == 0), stop=(kt == ND - 1))
                        nc.vector.tensor_copy(
                            p2_sb[:, mb * D + c0: mb * D + c0 + cw],
                            ps[:, :cw])
                for (c0, cw) in CH_D:   # P2 row d' = 768 (bias row)
                    ps = ps4.tile([P, 512], f32, name=f"p2r{c0}", tag="ps4")
                    for kt in range(ND):
                        mm(ps[0:1, :cw],
                           aqt_sb[:, kt * DA + 768: kt * DA + 769],
                           kv_sb[:, kt * D + c0: kt * D + c0 + cw],
                           start=(kt == 0), stop=(kt == ND - 1))
                    nc.vector.tensor_copy(p2_row[0:1, c0:c0 + cw],
                                          ps[0:1, :cw])

            # ---- Stage 5: out = x' P2 for this core's S-half ----
            with tc.tile_pool(name="xtp", bufs=1) as xtp, \
                 tc.tile_pool(name="osb", bufs=3) as osbp, \
                 tc.tile_pool(name="ps5", bufs=4, space="PSUM") as ps5:
                xt_sb = xtp.tile([P, ND * SH], f32, name="xt_sb", tag="xt_sb")
                for kt in range(ND):
                    nc.sync.dma_start(out=xt_sb[:, kt * SH:(kt + 1) * SH],
                                      in_=xt[kt * P:(kt + 1) * P, :])
                for sb in range(NT_SH):
                    o = osbp.tile([P, D], f32, name=f"o{sb}", tag="osb")
                    for (c0, cw) in CH_D:
                        ps = ps5.tile([P, 512], f32, name=f"ops{sb}_{c0}",
                                      tag="ps5")
                        for kt in range(ND):
                            mm(ps[:, :cw],
                               xt_sb[:, kt * SH + sb * P:
                                     kt * SH + (sb + 1) * P],
                               p2_sb[:, kt * D + c0: kt * D + c0 + cw],
                               start=(kt == 0), stop=False)
                        mm(ps[:, :cw], ones_row[0:1, 0:P],
                           p2_row[0:1, c0:c0 + cw], start=False, stop=True)
                        nc.vector.tensor_copy(o[:, c0:c0 + cw], ps[:, :cw])
                    nc.sync.dma_start(out=outd[sb * P:(sb + 1) * P, :],
                                      in_=o[:, :])

    nc.compile()
    return nc


def get_nc():
    if "nc" not in _CACHE:
        _CACHE["nc"] = _build_nc(**CONFIG)
    return _CACHE["nc"]


def make_in_maps(x, Wq, bq, Wk, bk, Wv, bv):
    x = np.asarray(x, dtype=np.float32)
    scale = np.float32(1.0 / math.sqrt(D))
    f32 = np.float32
    ak = np.ascontiguousarray(
        np.concatenate([np.asarray(Wk, f32).T, np.asarray(bk, f32)[None, :]], 0))
    av = np.ascontiguousarray(
        np.concatenate([np.asarray(Wv, f32).T, np.asarray(bv, f32)[None, :]], 0))
    aqt = np.ascontiguousarray(
        np.concatenate([np.asarray(Wq, f32), np.asarray(bq, f32)[:, None]], 1)
    ) * scale
    in_maps = []
    for core in range(N_CORES):
        b, h = core // 2, core % 2
        xa = np.ascontiguousarray(
            np.concatenate([x[b], np.ones((S, 1), f32)], 1))
        xt = np.ascontiguousarray(x[b, h * SH:(h + 1) * SH, :].T)
        in_maps.append({"xa": xa, "xt": xt, "ak": ak, "av": av, "aqt": aqt})
    return in_maps


def gather_out(results):
    out = np.empty((B, S, D), np.float32)
    for core in range(N_CORES):
        b, h = core // 2, core % 2
        out[b, h * SH:(h + 1) * SH] = results[core]["out"]
    return out


def run(in_maps, trace=False, **kwargs):
    from concourse import bass_utils
    nc = get_nc()
    return bass_utils.run_bass_kernel_spmd(nc, in_maps, list(range(N_CORES)),
                                           trace=trace, **kwargs)


def kernel(x, Wq, bq, Wk, bk, Wv, bv):
    in_maps = make_in_maps(x, Wq, bq, Wk, bk, Wv, bv)
    res = run(in_maps)
    return gather_out(res.results)


# revision 7
# speedup vs baseline: 2.8414x; 2.8414x over previous
"""Trainium2 Bass kernel for batched linear attention (no softmax).

Reference computation (per batch b):
    q = x Wq^T + bq ; k = x Wk^T + bk ; v = x Wv^T + bv
    out = (q k^T / sqrt(D)) v

Since there is no softmax, matmul associativity gives
    out = q (k^T v) / sqrt(D)
and with augmented matrices x' = [x | 1 | 0], Aw = [W^T ; b ; 0] (so
w = x' Aw):
    k^T v = Ak^T (x'^T x') Av = Ak^T G' Av
    out   = x' (Aq* (Ak^T G' Av))          with Aq* = Aq / sqrt(D)

which replaces the two S x S matmuls (dominant cost) with [D+2]-sized ones.
G' is symmetric, so only the upper trapezoid is computed on the PE and the
lower blocks are filled by PE transposes. The augmented dim is padded to 770
(ones column + zeros column) because fp32r matmuls require even access
pattern widths.

Sharding: 8 cores = 4 batches x 2 S-halves. Each core computes G' for its
full batch (pair-redundant) and the output projection only for its S-half.
All weight-layout prep (transposes, augmentation, scale folding) happens on
the host; the device does all O(S) FLOPs.

mm_dt config: "f32" = exact fp32 matmuls (4 PE cycles/row);
"f32r" = fp32 with 11-bit mantissa (TF32-like, 1 cycle/row at even widths).
"""

import math

import numpy as np

B, S, D = 4, 4096, 768
DA = D + 2          # augmented dim: ones col at 768, zeros col at 769
P = 128
SH = S // 2         # per-core sequence half
N_CORES = 8
NT_S = S // P       # 32 sequence tiles for G'
ND = D // P         # 6 blocks of 128 over D
NT_SH = SH // P     # 16 output row blocks
CH_D = [(0, 512), (512, 256)]    # free-dim chunks covering 768

CONFIG = {"mm_dt": "f32r", "sym_g": True}

_CACHE = {}


def _chunks(c0, c1, step=512):
    out = []
    while c0 < c1:
        w = min(step, c1 - c0)
        out.append((c0, w))
        c0 += w
    return out


def _build_nc(mm_dt="f32r", sym_g=True):
    import concourse.bacc as bacc
    import concourse.mybir as mybir
    import concourse.tile as tile
    from concourse.masks import make_identity

    f32 = mybir.dt.float32
    sb = {"f32": f32, "f32r": mybir.dt.float32r}[mm_dt]

    nc = bacc.Bacc("TRN2", target_bir_lowering=False, debug=False,
                   num_devices=N_CORES)

    xa_t = nc.dram_tensor("xa", [S, DA], sb, kind="ExternalInput")
    xt_t = nc.dram_tensor("xt", [D, SH], sb, kind="ExternalInput")
    ak_t = nc.dram_tensor("ak", [DA, D], sb, kind="ExternalInput")
    av_t = nc.dram_tensor("av", [DA, D], sb, kind="ExternalInput")
    aqt_t = nc.dram_tensor("aqt", [D, DA], sb, kind="ExternalInput")
    out_t = nc.dram_tensor("out", [SH, D], f32, kind="ExternalOutput")
    xa, xt, ak, av, aqt, outd = (t.ap() for t in
                                 (xa_t, xt_t, ak_t, av_t, aqt_t, out_t))

    def mm(ps, lh, rh, start, stop):
        nc.tensor.matmul(ps, lhsT=lh, rhs=rh, start=start, stop=stop)

    with tile.TileContext(nc) as tc:
        with tc.tile_pool(name="persist", bufs=1) as pp:
            # G' rows d'<768 stored as 6 partition-tiles side by side:
            # g_sb[p, t*DA + j] = G'[t*128 + p, j]
            g_sb = pp.tile([P, ND * DA], sb, name="g_sb", tag="g_sb")
            g_row = pp.tile([2, DA], sb, name="g_row", tag="g_row")
            at_sb = pp.tile([P, ND * D], sb, name="at_sb", tag="at_sb")
            at_row = pp.tile([2, D], sb, name="at_row", tag="at_row")
            kv_sb = pp.tile([P, ND * D], sb, name="kv_sb", tag="kv_sb")
            p2_sb = pp.tile([P, ND * D], sb, name="p2_sb", tag="p2_sb")
            p2_row = pp.tile([2, D], sb, name="p2_row", tag="p2_row")
            ident = pp.tile([P, P], sb, name="ident", tag="ident")
            ones2 = pp.tile([2, P], sb, name="ones2", tag="ones2")
            # f32 scratch for constants (memset/iota can't write f32r);
            # the DVE copy is the sanctioned fp32 -> fp32r rounder.
            idf = pp.tile([P, P], f32, name="idf", tag="idf")
            ones2f = pp.tile([2, P], f32, name="ones2f", tag="ones2f")
            zrow = pp.tile([2, DA], f32, name="zrow", tag="zrow")
            corner = pp.tile([1, 2], f32, name="corner", tag="corner")
            make_identity(nc, idf)
            nc.any.memset(ones2f[0:2, :], 0.0)
            nc.any.memset(ones2f[0:1, :], 1.0)
            nc.any.memset(zrow[0:2, :], 0.0)
            nc.any.memset(corner[0:1, 0:1], float(S))
            nc.any.memset(corner[0:1, 1:2], 0.0)
            nc.vector.tensor_copy(ident[:, :], idf[:, :])
            nc.vector.tensor_copy(ones2[0:2, :], ones2f[0:2, :])

            # ---- Stage 1: G' = x'^T x' ----
            with tc.tile_pool(name="xp", bufs=1) as xp, \
                 tc.tile_pool(name="gps", bufs=6, space="PSUM") as gpsp:
                x_tiles = []
                for i in range(NT_S):
                    t = xp.tile([P, DA], sb, name=f"x{i}", tag=f"x{i}")
                    nc.sync.dma_start(out=t[:, :], in_=xa[i * P:(i + 1) * P, :])
                    x_tiles.append(t)

                if sym_g:
                    # upper trapezoid only: rows md*128..+128, cols md*128..DA
                    jobs = []
                    for md in range(ND):
                        jobs += [(md, c0, cw) for (c0, cw)
                                 in _chunks(md * P, DA)]
                    passes = [[j for j in jobs if j[0] < 3],
                              [j for j in jobs if j[0] >= 3]]
                else:
                    passes = [[(md, c0, cw) for md in range(ND)]
                              for (c0, cw) in _chunks(0, DA)]

                for pass_jobs in passes:
                    pss = {}
                    for (md, c0, cw) in pass_jobs:
                        pss[(md, c0)] = gpsp.tile(
                            [P, 512], f32, name=f"gps_{md}_{c0}", tag="gps")
                    for st in range(NT_S):
                        for (md, c0, cw) in pass_jobs:
                            mm(pss[(md, c0)][:, :cw],
                               x_tiles[st][:, md * P:(md + 1) * P],
                               x_tiles[st][:, c0:c0 + cw],
                               start=(st == 0), stop=(st == NT_S - 1))
                    for (md, c0, cw) in pass_jobs:
                        nc.vector.tensor_copy(
                            g_sb[:, md * DA + c0: md * DA + c0 + cw],
                            pss[(md, c0)][:, :cw])

            # mirror lower-triangle blocks + 2-row tail [m ; 0]
            with tc.tile_pool(name="tps", bufs=4, space="PSUM") as tpsp:
                if sym_g:
                    for md in range(1, ND):
                        for nb in range(md):
                            pt = tpsp.tile([P, 512], sb,
                                           name=f"tm{md}_{nb}", tag="tps")
                            nc.tensor.matmul(
                                pt[:, 0:P],
                                lhsT=g_sb[:, nb * DA + md * P:
                                          nb * DA + (md + 1) * P],
                                rhs=ident[:, :], is_transpose=True,
                                start=True, stop=True)
                            nc.vector.tensor_copy(
                                g_sb[:, md * DA + nb * P:
                                     md * DA + (nb + 1) * P],
                                pt[:, 0:P])
                # g_row row 0 = [m | S | 0], row 1 = 0
                nc.vector.tensor_copy(g_row[0:2, :], zrow[0:2, :])
                for t in range(ND):
                    pr = tpsp.tile([P, 512], sb, name=f"tp{t}", tag="tps")
                    nc.tensor.matmul(
                        pr[0:1, 0:P],
                        lhsT=g_sb[:, t * DA + 768: t * DA + 769],
                        rhs=ident[:, :], is_transpose=True,
                        start=True, stop=True)
                    nc.vector.tensor_copy(g_row[0:1, t * P:(t + 1) * P],
                                          pr[0:1, 0:P])
                nc.vector.tensor_copy(g_row[0:1, 768:770], corner[0:1, 0:2])

            # ---- Stage 2: AT' = G' Ak  ([770, 768], f' on partitions) ----
            with tc.tile_pool(name="akp", bufs=1) as akp, \
                 tc.tile_pool(name="ps2", bufs=4, space="PSUM") as ps2:
                ak_sb = akp.tile([P, ND * D], sb, name="ak_sb", tag="ak_sb")
                ak_row = akp.tile([2, D], sb, name="ak_row", tag="ak_row")
                for kt in range(ND):
                    nc.sync.dma_start(out=ak_sb[:, kt * D:(kt + 1) * D],
                                      in_=ak[kt * P:(kt + 1) * P, :])
                nc.sync.dma_start(out=ak_row[0:2, :], in_=ak[768:770, :])
                for mb in range(ND):
                    for (c0, cw) in CH_D:
                        ps = ps2.tile([P, 512], f32, name=f"atps{mb}_{c0}",
                                      tag="atps")
                        for kt in range(ND + 1):
                            if kt < ND:
                                lh = g_sb[:, kt * DA + mb * P:
                                          kt * DA + (mb + 1) * P]
                                rh = ak_sb[:, kt * D + c0: kt * D + c0 + cw]
                            else:
                                lh = g_row[0:2, mb * P:(mb + 1) * P]
                                rh = ak_row[0:2, c0:c0 + cw]
                            mm(ps[:, :cw], lh, rh,
                               start=(kt == 0), stop=(kt == ND))
                        nc.vector.tensor_copy(
                            at_sb[:, mb * D + c0: mb * D + c0 + cw],
                            ps[:, :cw])
                for (c0, cw) in CH_D:   # AT' rows [768:770]
                    ps = ps2.tile([P, 512], f32, name=f"atr{c0}", tag="atps")
                    for kt in range(ND + 1):
                        if kt < ND:
                            lh = g_sb[:, kt * DA + 768: kt * DA + 770]
                            rh = ak_sb[:, kt * D + c0: kt * D + c0 + cw]
                        else:
                            lh = g_row[0:2, 768:770]
                            rh = ak_row[0:2, c0:c0 + cw]
                        mm(ps[0:2, :cw], lh, rh,
                           start=(kt == 0), stop=(kt == ND))
                    nc.vector.tensor_copy(at_row[0:2, c0:c0 + cw],
                                          ps[0:2, :cw])

            # ---- Stage 3: kv = AT'^T Av  ([768, 768], e on partitions) ----
            with tc.tile_pool(name="avp", bufs=1) as avp, \
                 tc.tile_pool(name="ps3", bufs=4, space="PSUM") as ps3:
                av_sb = avp.tile([P, ND * D], sb, name="av_sb", tag="av_sb")
                av_row = avp.tile([2, D], sb, name="av_row", tag="av_row")
                for kt in range(ND):
                    nc.sync.dma_start(out=av_sb[:, kt * D:(kt + 1) * D],
                                      in_=av[kt * P:(kt + 1) * P, :])
                nc.sync.dma_start(out=av_row[0:2, :], in_=av[768:770, :])
                for mb in range(ND):
                    for (c0, cw) in CH_D:
                        ps = ps3.tile([P, 512], f32, name=f"kvps{mb}_{c0}",
                                      tag="kvps")
                        for kt in range(ND + 1):
                            if kt < ND:
                                lh = at_sb[:, kt * D + mb * P:
                                           kt * D + (mb + 1) * P]
                                rh = av_sb[:, kt * D + c0: kt * D + c0 + cw]
                            else:
                                lh = at_row[0:2, mb * P:(mb + 1) * P]
                                rh = av_row[0:2, c0:c0 + cw]
                            mm(ps[:, :cw], lh, rh,
                               start=(kt == 0), stop=(kt == ND))
                        nc.vector.tensor_copy(
                            kv_sb[:, mb * D + c0: mb * D + c0 + cw],
                            ps[:, :cw])

            # ---- Stage 4: P2 = Aq* kv  ([770, 768], d' on partitions) ----
            with tc.tile_pool(name="aqp", bufs=1) as aqp, \
                 tc.tile_pool(name="ps4", bufs=4, space="PSUM") as ps4:
                aqt_sb = aqp.tile([P, ND * DA], sb, name="aqt_sb",
                                  tag="aqt_sb")
                for kt in range(ND):
                    nc.sync.dma_start(out=aqt_sb[:, kt * DA:(kt + 1) * DA],
                                      in_=aqt[kt * P:(kt + 1) * P, :])
                for mb in range(ND):
                    for (c0, cw) in CH_D:
                        ps = ps4.tile([P, 512], f32, name=f"p2ps{mb}_{c0}",
                                      tag="p2ps")
                        for kt in range(ND):
                            mm(ps[:, :cw],
                               aqt_sb[:, kt * DA + mb * P:
                                      kt * DA + (mb + 1) * P],
                               kv_sb[:, kt * D + c0: kt * D + c0 + cw],
                               start=(kt == 0), stop=(kt == ND - 1))
                        nc.vector.tensor_copy(
                            p2_sb[:, mb * D + c0: mb * D + c0 + cw],
                            ps[:, :cw])
                for (c0, cw) in CH_D:   # P2 rows [768:770] (bias row; zero)
                    ps = ps4.tile([P, 512], f32, name=f"p2r{c0}", tag="p2ps")
                    for kt in range(ND):
                        mm(ps[0:2, :cw],
                           aqt_sb[:, kt * DA + 768: kt * DA + 770],
                           kv_sb[:, kt * D + c0: kt * D + c0 + cw],
                           start=(kt == 0), stop=(kt == ND - 1))
                    nc.vector.tensor_copy(p2_row[0:2, c0:c0 + cw],
                                          ps[0:2, :cw])

            # ---- Stage 5: out = x' P2 for this core's S-half ----
            with tc.tile_pool(name="xtp", bufs=1) as xtp, \
                 tc.tile_pool(name="osb", bufs=3) as osbp, \
                 tc.tile_pool(name="ps5", bufs=4, space="PSUM") as ps5:
                xt_sb = xtp.tile([P, ND * SH], sb, name="xt_sb", tag="xt_sb")
                for kt in range(ND):
                    nc.sync.dma_start(out=xt_sb[:, kt * SH:(kt + 1) * SH],
                                      in_=xt[kt * P:(kt + 1) * P, :])
                for sbk in range(NT_SH):
                    o = osbp.tile([P, D], f32, name=f"o{sbk}", tag="osb")
                    for (c0, cw) in CH_D:
                        ps = ps5.tile([P, 512], f32, name=f"ops{sbk}_{c0}",
                                      tag="ops")
                        for kt in range(ND):
                            mm(ps[:, :cw],
                               xt_sb[:, kt * SH + sbk * P:
                                     kt * SH + (sbk + 1) * P],
                               p2_sb[:, kt * D + c0: kt * D + c0 + cw],
                               start=(kt == 0), stop=False)
                        mm(ps[:, :cw], ones2[0:2, 0:P],
                           p2_row[0:2, c0:c0 + cw], start=False, stop=True)
                        nc.vector.tensor_copy(o[:, c0:c0 + cw], ps[:, :cw])
                    nc.sync.dma_start(out=outd[sbk * P:(sbk + 1) * P, :],
                                      in_=o[:, :])

    nc.compile()
    return nc


def get_nc():
    if "nc" not in _CACHE:
        _CACHE["nc"] = _build_nc(**CONFIG)
    return _CACHE["nc"]


def make_in_maps(x, Wq, bq, Wk, bk, Wv, bv):
    x = np.asarray(x, dtype=np.float32)
    scale = np.float32(1.0 / math.sqrt(D))
    f32 = np.float32
    z1 = np.zeros((1, D), f32)
    ak = np.ascontiguousarray(np.concatenate(
        [np.asarray(Wk, f32).T, np.asarray(bk, f32)[None, :], z1], 0))
    av = np.ascontiguousarray(np.concatenate(
        [np.asarray(Wv, f32).T, np.asarray(bv, f32)[None, :], z1], 0))
    aqt = np.ascontiguousarray(np.concatenate(
        [np.asarray(Wq, f32), np.asarray(bq, f32)[:, None],
         np.zeros((D, 1), f32)], 1)) * scale
    in_maps = []
    for core in range(N_CORES):
        b, h = core // 2, core % 2
        xa = np.ascontiguousarray(np.concatenate(
            [x[b], np.ones((S, 1), f32), np.zeros((S, 1), f32)], 1))
        xt = np.ascontiguousarray(x[b, h * SH:(h + 1) * SH, :].T)
        in_maps.append({"xa": xa, "xt": xt, "ak": ak, "av": av, "aqt": aqt})
    return in_maps


def gather_out(results):
    out = np.empty((B, S, D), np.float32)
    for core in range(N_CORES):
        b, h = core // 2, core % 2
        out[b, h * SH:(h + 1) * SH] = results[core]["out"]
    return out


def run(in_maps, trace=False, **kwargs):
    from concourse import bass_utils
    nc = get_nc()
    return bass_utils.run_bass_kernel_spmd(nc, in_maps, list(range(N_CORES)),
                                           trace=trace, **kwargs)


def kernel(x, Wq, bq, Wk, bk, Wv, bv):
    in_maps = make_in_maps(x, Wq, bq, Wk, bk, Wv, bv)
    res = run(in_maps)
    return gather_out(res.results)


# revision 11
# speedup vs baseline: 3.5934x; 1.2647x over previous
"""Trainium2 Bass kernel for batched linear attention (no softmax).

Reference computation (per batch b):
    q = x Wq^T + bq ; k = x Wk^T + bk ; v = x Wv^T + bv
    out = (q k^T / sqrt(D)) v

Since there is no softmax, matmul associativity gives
    out = q (k^T v) / sqrt(D)
and with augmented matrices x' = [x | 1 | 0], Aw = [W^T ; b ; 0] (so
w = x' Aw):
    k^T v = Ak^T (x'^T x') Av = Ak^T G' Av
    out   = x' (Aq* (Ak^T G' Av))          with Aq* = Aq / sqrt(D)

which replaces the two S x S matmuls (dominant cost) with [D+2]-sized ones.
G' is symmetric, so only the upper trapezoid is computed on the PE and the
lower blocks are filled by PE transposes. The augmented dim is padded to 770
(ones column + zeros column) because fp32r matmuls require even access
pattern widths.

Sharding: 8 cores = 4 batches x 2 S-halves. Each core computes G' for its
full batch (pair-redundant) and the output projection only for its S-half.
All weight-layout prep (transposes, augmentation, scale folding) happens on
the host; the device does all O(S) FLOPs.

mm_dt config: "f32" = exact fp32 matmuls (4 PE cycles/row);
"f32r" = fp32 with 11-bit mantissa (TF32-like, 1 cycle/row at even widths).
"""

import math

import numpy as np

B, S, D = 4, 4096, 768
DA = D + 2          # augmented dim: ones col at 768, zeros col at 769
P = 128
SH = S // 2         # per-core sequence half
N_CORES = 8
NT_S = S // P       # 32 sequence tiles for G'
ND = D // P         # 6 blocks of 128 over D
NT_SH = SH // P     # 16 output row blocks
CH_D = [(0, 512), (512, 256)]    # free-dim chunks covering 768

CONFIG = {"mm_dt": "f32r", "sym_g": True}

_CACHE = {}


def _chunks(c0, c1, step=512):
    out = []
    while c0 < c1:
        w = min(step, c1 - c0)
        out.append((c0, w))
        c0 += w
    return out


def _build_nc(mm_dt="f32r", sym_g=True):
    import concourse.bacc as bacc
    import concourse.mybir as mybir
    import concourse.tile as tile
    from concourse.masks import make_identity

    f32 = mybir.dt.float32
    sb = {"f32": f32, "f32r": mybir.dt.float32r}[mm_dt]

    nc = bacc.Bacc("TRN2", target_bir_lowering=False, debug=False,
                   num_devices=N_CORES)

    xa_t = nc.dram_tensor("xa", [S, DA], sb, kind="ExternalInput")
    xt_t = nc.dram_tensor("xt", [D, SH], sb, kind="ExternalInput")
    ak_t = nc.dram_tensor("ak", [DA, D], sb, kind="ExternalInput")
    av_t = nc.dram_tensor("av", [DA, D], sb, kind="ExternalInput")
    aqt_t = nc.dram_tensor("aqt", [D, DA], sb, kind="ExternalInput")
    out_t = nc.dram_tensor("out", [SH, D], f32, kind="ExternalOutput")
    xa, xt, ak, av, aqt, outd = (t.ap() for t in
                                 (xa_t, xt_t, ak_t, av_t, aqt_t, out_t))

    def mm(ps, lh, rh, start, stop):
        nc.tensor.matmul(ps, lhsT=lh, rhs=rh, start=start, stop=stop)

    from contextlib import ExitStack
    with tile.TileContext(nc) as tc:
        with tc.tile_pool(name="persist", bufs=1) as pp:
            # G' rows d'<768 stored as 6 partition-tiles side by side:
            # g_sb[p, t*DA + j] = G'[t*128 + p, j]
            g_sb = pp.tile([P, ND * DA], sb, name="g_sb", tag="g_sb")
            g_row = pp.tile([2, DA], sb, name="g_row", tag="g_row")
            ident = pp.tile([P, P], sb, name="ident", tag="ident")
            ones2 = pp.tile([2, P], sb, name="ones2", tag="ones2")
            # f32 scratch for constants (memset/iota can't write f32r);
            # the DVE copy is the sanctioned fp32 -> fp32r rounder.
            idf = pp.tile([P, P], f32, name="idf", tag="idf")
            ones2f = pp.tile([2, P], f32, name="ones2f", tag="ones2f")
            zrow = pp.tile([2, DA], f32, name="zrow", tag="zrow")
            corner = pp.tile([1, 2], f32, name="corner", tag="corner")
            make_identity(nc, idf)
            nc.any.memset(ones2f[0:2, :], 0.0)
            nc.any.memset(ones2f[0:1, :], 1.0)
            nc.any.memset(zrow[0:2, :], 0.0)
            nc.any.memset(corner[0:1, 0:1], float(S))
            nc.any.memset(corner[0:1, 1:2], 0.0)
            nc.vector.tensor_copy(ident[:, :], idf[:, :])
            nc.vector.tensor_copy(ones2[0:2, :], ones2f[0:2, :])

            # Weight pools open for the whole kernel; their DMAs are
            # emitted AFTER the x loads so they fill the DMA tail of the
            # G' phase instead of delaying it.
            es = ExitStack()
            wp = es.enter_context(tc.tile_pool(name="wp", bufs=1))
            aqp = es.enter_context(tc.tile_pool(name="aqp", bufs=1))
            ak_sb = wp.tile([P, ND * D], sb, name="ak_sb", tag="ak_sb")
            ak_row = wp.tile([2, D], sb, name="ak_row", tag="ak_row")
            av_sb = wp.tile([P, ND * D], sb, name="av_sb", tag="av_sb")
            av_row = wp.tile([2, D], sb, name="av_row", tag="av_row")
            aqt_sb = aqp.tile([P, ND * DA], sb, name="aqt_sb", tag="aqt_sb")

            # ---- Stage 1: G' = x'^T x' ----
            with tc.tile_pool(name="xp", bufs=1) as xp, \
                 tc.tile_pool(name="gps", bufs=8, space="PSUM") as gpsp:
                x_tiles = []
                for i in range(NT_S):
                    t = xp.tile([P, DA], sb, name=f"x{i}", tag=f"x{i}")
                    nc.sync.dma_start(out=t[:, :], in_=xa[i * P:(i + 1) * P, :])
                    x_tiles.append(t)
                for kt in range(ND):
                    nc.sync.dma_start(out=ak_sb[:, kt * D:(kt + 1) * D],
                                      in_=ak[kt * P:(kt + 1) * P, :])
                nc.sync.dma_start(out=ak_row[0:2, :], in_=ak[768:770, :])
                for kt in range(ND):
                    nc.sync.dma_start(out=av_sb[:, kt * D:(kt + 1) * D],
                                      in_=av[kt * P:(kt + 1) * P, :])
                nc.sync.dma_start(out=av_row[0:2, :], in_=av[768:770, :])
                for kt in range(ND):
                    nc.sync.dma_start(out=aqt_sb[:, kt * DA:(kt + 1) * DA],
                                      in_=aqt[kt * P:(kt + 1) * P, :])

                if sym_g:
                    # upper trapezoid only: rows md*128..+128, cols md*128..DA
                    jobs = []
                    for md in range(ND):
                        jobs += [(md, c0, cw) for (c0, cw)
                                 in _chunks(md * P, DA)]
                    passes = [[j for j in jobs if j[0] < 3],
                              [j for j in jobs if j[0] >= 3]]
                else:
                    passes = [[(md, c0, cw) for md in range(ND)]
                              for (c0, cw) in _chunks(0, DA)]

                for pass_jobs in passes:
                    pss = {}
                    for (md, c0, cw) in pass_jobs:
                        pss[(md, c0)] = gpsp.tile(
                            [P, 512], f32, name=f"gps_{md}_{c0}", tag="gps")
                    for st in range(NT_S):
                        for (md, c0, cw) in pass_jobs:
                            mm(pss[(md, c0)][:, :cw],
                               x_tiles[st][:, md * P:(md + 1) * P],
                               x_tiles[st][:, c0:c0 + cw],
                               start=(st == 0), stop=(st == NT_S - 1))
                    for (md, c0, cw) in pass_jobs:
                        nc.vector.tensor_copy(
                            g_sb[:, md * DA + c0: md * DA + c0 + cw],
                            pss[(md, c0)][:, :cw])

            # mirror lower-triangle blocks + 2-row tail [m ; 0]
            with tc.tile_pool(name="tps", bufs=4, space="PSUM") as tpsp:
                if sym_g:
                    for md in range(1, ND):
                        for nb in range(md):
                            pt = tpsp.tile([P, 512], sb,
                                           name=f"tm{md}_{nb}", tag="tps")
                            nc.tensor.matmul(
                                pt[:, 0:P],
                                lhsT=g_sb[:, nb * DA + md * P:
                                          nb * DA + (md + 1) * P],
                                rhs=ident[:, :], is_transpose=True,
                                start=True, stop=True)
                            nc.vector.tensor_copy(
                                g_sb[:, md * DA + nb * P:
                                     md * DA + (nb + 1) * P],
                                pt[:, 0:P])
                # g_row row 0 = [m | S | 0], row 1 = 0
                nc.vector.tensor_copy(g_row[0:2, :], zrow[0:2, :])
                for t in range(ND):
                    pr = tpsp.tile([P, 512], sb, name=f"tp{t}", tag="tps")
                    nc.tensor.matmul(
                        pr[0:1, 0:P],
                        lhsT=g_sb[:, t * DA + 768: t * DA + 769],
                        rhs=ident[:, :], is_transpose=True,
                        start=True, stop=True)
                    nc.vector.tensor_copy(g_row[0:1, t * P:(t + 1) * P],
                                          pr[0:1, 0:P])
                nc.vector.tensor_copy(g_row[0:1, 768:770], corner[0:1, 0:2])

            # mats tiles (on-chip intermediates) + xt prefetch for stage 5
            mats = es.enter_context(tc.tile_pool(name="mats", bufs=1))
            xtp = es.enter_context(tc.tile_pool(name="xtp", bufs=1))
            at_sb = mats.tile([P, ND * D], sb, name="at_sb", tag="big")
            at_row = mats.tile([2, D], sb, name="at_row", tag="rowb")
            kv_sb = mats.tile([P, ND * D], sb, name="kv_sb", tag="kv_sb")
            xt_sb = xtp.tile([P, ND * SH], sb, name="xt_sb", tag="xt_sb")
            for kt in range(ND):
                nc.sync.dma_start(out=xt_sb[:, kt * SH:(kt + 1) * SH],
                                  in_=xt[kt * P:(kt + 1) * P, :])

            # ---- Stage 2: AT' = G' Ak  ([770, 768], f' on partitions) ----
            with tc.tile_pool(name="ps2", bufs=4, space="PSUM") as ps2:
                for mb in range(ND):
                    for (c0, cw) in CH_D:
                        ps = ps2.tile([P, 512], f32, name=f"atps{mb}_{c0}",
                                      tag="atps")
                        for kt in range(ND + 1):
                            if kt < ND:
                                lh = g_sb[:, kt * DA + mb * P:
                                          kt * DA + (mb + 1) * P]
                                rh = ak_sb[:, kt * D + c0: kt * D + c0 + cw]
                            else:
                                lh = g_row[0:2, mb * P:(mb + 1) * P]
                                rh = ak_row[0:2, c0:c0 + cw]
                            mm(ps[:, :cw], lh, rh,
                               start=(kt == 0), stop=(kt == ND))
                        nc.vector.tensor_copy(
                            at_sb[:, mb * D + c0: mb * D + c0 + cw],
                            ps[:, :cw])
                for (c0, cw) in CH_D:   # AT' rows [768:770]
                    ps = ps2.tile([P, 512], f32, name=f"atr{c0}", tag="atps")
                    for kt in range(ND + 1):
                        if kt < ND:
                            lh = g_sb[:, kt * DA + 768: kt * DA + 770]
                            rh = ak_sb[:, kt * D + c0: kt * D + c0 + cw]
                        else:
                            lh = g_row[0:2, 768:770]
                            rh = ak_row[0:2, c0:c0 + cw]
                        mm(ps[0:2, :cw], lh, rh,
                           start=(kt == 0), stop=(kt == ND))
                    nc.vector.tensor_copy(at_row[0:2, c0:c0 + cw],
                                          ps[0:2, :cw])

                # ---- Stage 3: kv = AT'^T Av ([768, 768]) ----
                ps3 = ps2
                for mb in range(ND):
                    for (c0, cw) in CH_D:
                        ps = ps3.tile([P, 512], f32, name=f"kvps{mb}_{c0}",
                                      tag="kvps")
                        for kt in range(ND + 1):
                            if kt < ND:
                                lh = at_sb[:, kt * D + mb * P:
                                           kt * D + (mb + 1) * P]
                                rh = av_sb[:, kt * D + c0: kt * D + c0 + cw]
                            else:
                                lh = at_row[0:2, mb * P:(mb + 1) * P]
                                rh = av_row[0:2, c0:c0 + cw]
                            mm(ps[:, :cw], lh, rh,
                               start=(kt == 0), stop=(kt == ND))
                        nc.vector.tensor_copy(
                            kv_sb[:, mb * D + c0: mb * D + c0 + cw],
                            ps[:, :cw])

            # ---- Stage 4: P2 = Aq* kv  ([770, 768], d' on partitions) ----
            with tc.tile_pool(name="ps4", bufs=4, space="PSUM") as ps4:
                p2_sb = mats.tile([P, ND * D], sb, name="p2_sb", tag="big")
                p2_row = mats.tile([2, D], sb, name="p2_row", tag="rowb")
                for mb in range(ND):
                    for (c0, cw) in CH_D:
                        ps = ps4.tile([P, 512], f32, name=f"p2ps{mb}_{c0}",
                                      tag="p2ps")
                        for kt in range(ND):
                            mm(ps[:, :cw],
                               aqt_sb[:, kt * DA + mb * P:
                                      kt * DA + (mb + 1) * P],
                               kv_sb[:, kt * D + c0: kt * D + c0 + cw],
                               start=(kt == 0), stop=(kt == ND - 1))
                        nc.vector.tensor_copy(
                            p2_sb[:, mb * D + c0: mb * D + c0 + cw],
                            ps[:, :cw])
                for (c0, cw) in CH_D:   # P2 rows [768:770] (bias row; zero)
                    ps = ps4.tile([P, 512], f32, name=f"p2r{c0}", tag="p2ps")
                    for kt in range(ND):
                        mm(ps[0:2, :cw],
                           aqt_sb[:, kt * DA + 768: kt * DA + 770],
                           kv_sb[:, kt * D + c0: kt * D + c0 + cw],
                           start=(kt == 0), stop=(kt == ND - 1))
                    nc.vector.tensor_copy(p2_row[0:2, c0:c0 + cw],
                                          ps[0:2, :cw])

            # ---- Stage 5: out = x' P2 for this core's S-half ----
            with tc.tile_pool(name="osb", bufs=3) as osbp, \
                 tc.tile_pool(name="ps5", bufs=4, space="PSUM") as ps5:
                # broadcast the bias row P2[768] across 128 partitions once;
                # each output block then adds it during PSUM eviction on DVE
                biasb = osbp.tile([P, D], f32, name="biasb", tag="biasb")
                for (c0, cw) in CH_D:
                    ps = ps5.tile([P, 512], f32, name=f"bps{c0}", tag="ops")
                    mm(ps[:, :cw], ones2[0:2, 0:P], p2_row[0:2, c0:c0 + cw],
                       start=True, stop=True)
                    nc.vector.tensor_copy(biasb[:, c0:c0 + cw], ps[:, :cw])
                for sbk in range(NT_SH):
                    o = osbp.tile([P, D], f32, name=f"o{sbk}", tag="osb")
                    for (c0, cw) in CH_D:
                        ps = ps5.tile([P, 512], f32, name=f"ops{sbk}_{c0}",
                                      tag="ops")
                        for kt in range(ND):
                            mm(ps[:, :cw],
                               xt_sb[:, kt * SH + sbk * P:
                                     kt * SH + (sbk + 1) * P],
                               p2_sb[:, kt * D + c0: kt * D + c0 + cw],
                               start=(kt == 0), stop=(kt == ND - 1))
                        nc.vector.tensor_add(o[:, c0:c0 + cw], ps[:, :cw],
                                             biasb[:, c0:c0 + cw])
                    nc.sync.dma_start(out=outd[sbk * P:(sbk + 1) * P, :],
                                      in_=o[:, :])
            es.close()

    nc.compile()
    return nc


def get_nc():
    if "nc" not in _CACHE:
        _CACHE["nc"] = _build_nc(**CONFIG)
    return _CACHE["nc"]


def make_in_maps(x, Wq, bq, Wk, bk, Wv, bv):
    x = np.asarray(x, dtype=np.float32)
    scale = np.float32(1.0 / math.sqrt(D))
    f32 = np.float32
    z1 = np.zeros((1, D), f32)
    ak = np.ascontiguousarray(np.concatenate(
        [np.asarray(Wk, f32).T, np.asarray(bk, f32)[None, :], z1], 0))
    av = np.ascontiguousarray(np.concatenate(
        [np.asarray(Wv, f32).T, np.asarray(bv, f32)[None, :], z1], 0))
    aqt = np.ascontiguousarray(np.concatenate(
        [np.asarray(Wq, f32), np.asarray(bq, f32)[:, None],
         np.zeros((D, 1), f32)], 1)) * scale
    in_maps = []
    for core in range(N_CORES):
        b, h = core // 2, core % 2
        xa = np.ascontiguousarray(np.concatenate(
            [x[b], np.ones((S, 1), f32), np.zeros((S, 1), f32)], 1))
        xt = np.ascontiguousarray(x[b, h * SH:(h + 1) * SH, :].T)
        in_maps.append({"xa": xa, "xt": xt, "ak": ak, "av": av, "aqt": aqt})
    return in_maps


def gather_out(results):
    out = np.empty((B, S, D), np.float32)
    for core in range(N_CORES):
        b, h = core // 2, core % 2
        out[b, h * SH:(h + 1) * SH] = results[core]["out"]
    return out


def run(in_maps, trace=False, **kwargs):
    from concourse import bass_utils
    nc = get_nc()
    return bass_utils.run_bass_kernel_spmd(nc, in_maps, list(range(N_CORES)),
                                           trace=trace, **kwargs)


def kernel(x, Wq, bq, Wk, bk, Wv, bv):
    in_maps = make_in_maps(x, Wq, bq, Wk, bk, Wv, bv)
    res = run(in_maps)
    return gather_out(res.results)
